# revision 14
# baseline (speedup 1.0000x reference)
"""BiConvLSTM kernel for one TRN2 chip (8 NeuronCores) — v2.

Strategy (8-way model parallelism over LSTM gate rows, pipelined AllGather):
  - Each core owns 288 hidden units; W_ih/W_hh column-slices live in SBUF.
  - All 4 gates are packed into wide moving operands: per k-tile the gate
    matmul is 3 MMs of N=512/512/128 (unit-groups of 128/128/32 units with
    gate column order i|f|o|g), instead of 4x N=288 -> fewer LDWEIGHTS
    stalls, near the bf16 streaming rate of the PE.
  - The x-projection (+bias) for all 128 (t, sample) columns is computed
    once; a per-step 0/1 selector matrix SEL_s folds the right rows into
    each gate PSUM as one extra matmul (no DVE add, no staging DMAs).
  - Hidden units are k-mapped: units of half A land in hT k-tiles 0..7,
    half B in k-tiles 8..17.  Each step runs TWO AllGathers (A then B);
    the next step's matmuls consume k 0..7 first, so AG_B may land ~3.4us
    into the next step.  AG latency hides under real matmuls (no dummies).
  - Epilogue: one sigmoid over [i|f|o], one tanh for g, c/h updates on DVE;
    h is produced in bf16, PE-transposed straight into a bf16 PSUM tile and
    DMA'd to the collective input buffer.

Sequence-row layout (row m of the 16-row state tiles):
  m 0..3   fwd samples 0,2,4,6;  m 4..7   bwd samples 0,2,4,6
  m 8..11  fwd samples 1,3,5,7;  m 12..15 bwd samples 1,3,5,7
This makes the reference's stack/reshape channel pairing (row b', b'+8).
"""

import sys

if "/opt/trn_rl_repo" not in sys.path:
    sys.path.append("/opt/trn_rl_repo")

import ml_dtypes
import numpy as np

from concourse import bacc, bass_utils, mybir, tile
from concourse.tile import add_dep_helper

B, T, H, W = 8, 16, 48, 48
HW = H * W              # 2304
NSEQ = 16               # 8 samples x 2 directions
NC = 8                  # cores
S = HW // NC            # 288 hidden units per core
G4 = 4 * S              # 1152 gate columns per core
KT = HW // 128          # 18 k-tiles
KTA = KT + 1            # +1 tile holding the bias/ones row
GO = [0, 1, 3, 2]       # column-position -> gate (i, f, o, g)
GW = [128, 32, 128]     # unit-group widths (A=128+32 units, B=128)
GBASE = [0, 512, 640]   # gate-column base per group
LBASE = [0, 128, 160]   # local-unit base per group
SAMP_ORDER = [0, 2, 4, 6, 1, 3, 5, 7]
F32 = mybir.dt.float32
BF16 = mybir.dt.bfloat16
RG = [list(range(NC))]


def _build(w0: float, w1: float, cb: float):
    nc = bacc.Bacc("TRN2", target_bir_lowering=False, debug=False, num_devices=NC)

    xT_d = nc.dram_tensor("xT", [128, KTA, 128], BF16, kind="ExternalInput")
    wih_d = nc.dram_tensor("wih", [128, KTA, G4], BF16, kind="ExternalInput")
    whh_d = nc.dram_tensor("whh", [128, KT, G4], BF16, kind="ExternalInput")
    sel_d = nc.dram_tensor("sel", [128, T, NSEQ], BF16, kind="ExternalInput")
    eye_d = nc.dram_tensor("eye16", [16, 16], BF16, kind="ExternalInput")
    out_d = nc.dram_tensor("out", [B, S], F32, kind="ExternalOutput")

    SIG = mybir.ActivationFunctionType.Sigmoid
    TANH = mybir.ActivationFunctionType.Tanh
    ADD = mybir.AluOpType.add
    MULT = mybir.AluOpType.mult
    MAX = mybir.AluOpType.max

    with tile.TileContext(nc) as tc:
        with (
            tc.tile_pool(name="const", bufs=1) as constp,
            tc.tile_pool(name="state", bufs=2) as statep,
            tc.tile_pool(name="ew", bufs=2) as ewp,
            tc.tile_pool(name="pg", bufs=2, space="PSUM") as pgp,
            tc.tile_pool(name="pt", bufs=1, space="PSUM") as ptp,
            tc.tile_pool(name="px", bufs=1, space="PSUM") as pxp,
            tc.tile_pool(name="dram", bufs=2, space="DRAM") as dp,
        ):
            # throwaway collective: absorbs the ncfw first-call warmup while
            # the weight DMAs stream in
            warm_in = dp.tile([16, NSEQ], BF16, tag="warmin")
            warm_out = dp.tile([128, NSEQ], BF16, addr_space="Shared", tag="warmout")
            warm_ag = nc.gpsimd.collective_compute(
                "AllGather", mybir.AluOpType.bypass,
                ins=[warm_in.opt()], outs=[warm_out.opt()], replica_groups=RG,
            )

            eye_sb = constp.tile([16, 16], BF16, tag="eye")
            nc.scalar.dma_start(eye_sb[:, :], eye_d[:, :])
            sel_sb = constp.tile([128, T, NSEQ], BF16, tag="sel")
            nc.scalar.dma_start(sel_sb[:, :, :], sel_d[:, :, :])
            # touch sigmoid/tanh so the ACT tables load off the critical path
            actwarm = constp.tile([16, 16], F32, tag="actwarm")
            nc.scalar.activation(actwarm[:, :], eye_sb[:, :], SIG)
            nc.scalar.activation(actwarm[:, :], eye_sb[:, :], TANH)

            xT_sb = constp.tile([128, KTA, 128], BF16, tag="xT")
            nc.sync.dma_start(xT_sb[:, 0:10, :], xT_d[:, 0:10, :])
            nc.scalar.dma_start(xT_sb[:, 10:KTA, :], xT_d[:, 10:KTA, :])

            # stream W_ih (k-ascending; xproj chases these), then W_hh
            wih_sb = constp.tile([128, KTA, G4], BF16, tag="wih")
            whh_sb = constp.tile([128, KT, G4], BF16, tag="whh")
            # gpsimd carries ONLY collectives: any weight DMA queued there can
            # get scheduled ahead of the warmup AG's doorbell and delay the
            # ~25-50us ncfw init that gates step 0's first real AllGather
            qs = [nc.sync, nc.scalar]
            wih_chunks = [(k, min(k + 2, KTA)) for k in range(0, KTA, 2)]
            for i, (k0, k1) in enumerate(wih_chunks):
                qs[i % 2].dma_start(wih_sb[:, k0:k1, :], wih_d[:, k0:k1, :])
            def stream_whh():
                # emitted at the END of step 0's body: keeps these issue-waits
                # behind step 0's scalar/sync work in the engine queues (whh
                # is only consumed from step 1 onward)
                whh_chunks = [(k, min(k + 2, KT)) for k in range(0, KT, 2)]
                for i, (k0, k1) in enumerate(whh_chunks):
                    qs[i % 2].dma_start(whh_sb[:, k0:k1, :], whh_d[:, k0:k1, :])

            # PE instruction chain: explicit deps pin the PE issue order
            prev_pe = None

            def pe(instr):
                nonlocal prev_pe
                if prev_pe is not None:
                    add_dep_helper(instr.ins, prev_pe.ins, False, reason="pe order")
                prev_pe = instr
                return instr

            # ---- xproj[n, c] = [xT; ones].T @ [W_ih.T; bias], all 128 cols
            xproj_bf = constp.tile([128, G4], BF16, tag="xproj")
            px = pxp.tile([128, 512], F32, tag="px")
            for g in range(3):
                cs = slice(GBASE[g], GBASE[g] + 4 * GW[g])
                for k in range(KTA):
                    pe(nc.tensor.matmul(
                        px[:, 0: 4 * GW[g]],
                        lhsT=xT_sb[:, k, :],
                        rhs=wih_sb[:, k, cs],
                        start=(k == 0),
                        stop=(k == KTA - 1),
                    ))
                nc.vector.tensor_copy(xproj_bf[:, cs], px[:, 0: 4 * GW[g]])

            c_prev = None
            h_last = None
            hT_prev = None

            def alloc_pg(s):
                return [
                    pgp.tile([NSEQ, 4 * GW[g]], F32, tag=f"pg{g}", name=f"pg{g}_{s}")
                    for g in range(3)
                ]

            def xsel_mms(pg, s):
                for g in range(3):
                    cs = slice(GBASE[g], GBASE[g] + 4 * GW[g])
                    pe(nc.tensor.matmul(
                        pg[g][:, :], lhsT=sel_sb[:, s, :], rhs=xproj_bf[:, cs],
                        start=True, stop=(s == 0),
                    ))

            def epi(pg, g, s, c_new, h_new):
                w = GW[g]
                lb = LBASE[g]
                sg = ewp.tile([NSEQ, 3 * w], F32, tag=f"sg{g}", name=f"sg{g}_{s}")
                tg = ewp.tile([NSEQ, w], F32, tag=f"tg{g}", name=f"tg{g}_{s}")
                nc.scalar.activation(tg[:, :], pg[g][:, 3 * w: 4 * w], TANH)
                nc.scalar.activation(sg[:, :], pg[g][:, 0: 3 * w], SIG)
                cslc = c_new[:, lb: lb + w]
                if s == 0:
                    nc.vector.tensor_tensor(cslc, sg[:, 0:w], tg[:, :], MULT)
                else:
                    m1 = ewp.tile([NSEQ, w], F32, tag=f"m1{g}", name=f"m1{g}_{s}")
                    fc = ewp.tile([NSEQ, w], F32, tag=f"fc{g}", name=f"fc{g}_{s}")
                    nc.vector.tensor_tensor(m1[:, :], sg[:, 0:w], tg[:, :], MULT)
                    nc.vector.tensor_tensor(
                        fc[:, :], sg[:, w: 2 * w], c_prev[:, lb: lb + w], MULT
                    )
                    nc.vector.tensor_tensor(cslc, fc[:, :], m1[:, :], ADD)
                tc_ = ewp.tile([NSEQ, w], F32, tag=f"tc{g}", name=f"tc{g}_{s}")
                nc.scalar.activation(tc_[:, :], cslc, TANH)
                nc.vector.tensor_tensor(
                    h_new[:, lb: lb + w], sg[:, 2 * w: 3 * w], tc_[:, :], MULT
                )

            # s=0 selector matmuls (gates = xproj only)
            pg_cur = alloc_pg(0)
            xsel_mms(pg_cur, 0)

            for s in range(T):
                pg = pg_cur
                c_new = statep.tile([NSEQ, S], F32, tag="c", name=f"c{s}")
                h_new = statep.tile([NSEQ, S], BF16, tag="h", name=f"h{s}")
                csl = [slice(GBASE[g], GBASE[g] + 4 * GW[g]) for g in range(3)]

                if s > 0:
                    # phase1: all groups' k0..9 (needs only AG_A of step s-1);
                    # long enough to cover AG_B's later arrival
                    for g in range(3):
                        for k in range(10):
                            pe(nc.tensor.matmul(
                                pg[g][:, :], lhsT=hT_prev[:, k, :],
                                rhs=whh_sb[:, k, csl[g]], start=False, stop=False,
                            ))
                    # phase2: k10..17, A-half groups first so AG_A goes early
                    for g in range(3):
                        for k in range(10, KT):
                            pe(nc.tensor.matmul(
                                pg[g][:, :], lhsT=hT_prev[:, k, :],
                                rhs=whh_sb[:, k, csl[g]], start=False,
                                stop=(k == KT - 1),
                            ))
                        epi(pg, g, s, c_new, h_new)
                else:
                    for g in range(3):
                        epi(pg, g, s, c_new, h_new)
                c_prev = c_new
                h_last = h_new

                if s < T - 1:
                    tp = ptp.tile([128, 48], BF16, tag="tp", name=f"tp{s}")
                    hts = ewp.tile([128, 48], BF16, tag="hts", name=f"hts{s}")
                    ccinA = dp.tile([160, NSEQ], BF16, tag="ccinA", name=f"ccinA{s}")
                    ccinB = dp.tile([128, NSEQ], BF16, tag="ccinB", name=f"ccinB{s}")
                    pe(nc.tensor.transpose(tp[:, 0:16], h_new[:, 0:128], eye_sb[:, :]))
                    nc.vector.tensor_copy(hts[:, 0:16], tp[:, 0:16])
                    nc.sync.dma_start(ccinA[0:128, :], hts[:, 0:16])
                    pe(nc.tensor.transpose(tp[0:32, 16:32], h_new[:, 128:160], eye_sb[:, :]))
                    nc.vector.tensor_copy(hts[0:32, 16:32], tp[0:32, 16:32])
                    nc.scalar.dma_start(ccinA[128:160, :], hts[0:32, 16:32])
                    pe(nc.tensor.transpose(tp[:, 32:48], h_new[:, 160:288], eye_sb[:, :]))
                    nc.vector.tensor_copy(hts[:, 32:48], tp[:, 32:48])
                    nc.sync.dma_start(ccinB[:, :], hts[:, 32:48])
                    ccoutA = dp.tile(
                        [128, 10, NSEQ], BF16, addr_space="Shared",
                        tag="ccoutA", name=f"ccoutA{s}",
                    )
                    ccoutB = dp.tile(
                        [128, 8, NSEQ], BF16, addr_space="Shared",
                        tag="ccoutB", name=f"ccoutB{s}",
                    )
                    agA = nc.gpsimd.collective_compute(
                        "AllGather", mybir.AluOpType.bypass,
                        ins=[ccinA.opt()], outs=[ccoutA.opt()], replica_groups=RG,
                    )
                    agB = nc.gpsimd.collective_compute(
                        "AllGather", mybir.AluOpType.bypass,
                        ins=[ccinB.opt()], outs=[ccoutB.opt()], replica_groups=RG,
                    )
                    add_dep_helper(agB.ins, agA.ins, False, reason="AG order")
                    # next step's selector matmuls + warm-keeper dummies fill
                    # the PE while the AllGathers run
                    pg_cur = alloc_pg(s + 1)
                    xsel_mms(pg_cur, s + 1)
                    for di in range(12):
                        pe(nc.tensor.matmul(
                            px[0:NSEQ, 0:512], lhsT=sel_sb[:, s, :],
                            rhs=xproj_bf[:, 0:512], start=True, stop=True,
                        ))
                    hT_new = statep.tile([128, KT, NSEQ], BF16, tag="hT", name=f"hT{s}")
                    nc.sync.dma_start(hT_new[:, 0:10, :], ccoutA[:, :, :])
                    nc.sync.dma_start(hT_new[:, 10:14, :], ccoutB[:, 0:4, :])
                    nc.sync.dma_start(hT_new[:, 14:KT, :], ccoutB[:, 4:8, :])
                    hT_prev = hT_new
                    if s == 0:
                        stream_whh()

            # ---- epilogue: y[b'] = leaky(w0*tanh(h[b']) + w1*tanh(h[b'+8]) + cb)
            th = ewp.tile([NSEQ, S], F32, tag="th")
            nc.scalar.activation(th[:, :], h_last[:, :], TANH)
            thb = ewp.tile([B, S], F32, tag="thb")
            nc.sync.dma_start(thb[:, :], th[8:16, :])
            yb = ewp.tile([B, S], F32, tag="yb")
            nc.vector.tensor_scalar(yb[:, :], thb[:, :], w1, cb, MULT, ADD)
            yc = ewp.tile([B, S], F32, tag="yc")
            nc.vector.scalar_tensor_tensor(yc[:, :], th[0:8, :], w0, yb[:, :], MULT, ADD)
            ye = ewp.tile([B, S], F32, tag="ye")
            nc.vector.scalar_tensor_tensor(ye[:, :], yc[:, :], 0.01, yc[:, :], MULT, MAX)
            nc.sync.dma_start(out_d[:, :], ye[:, :])

    nc.compile()
    return nc


# ---------------- host-side layout prep ----------------

def _unit_map():
    """local unit l -> (k, a); u = 128*k + 16*r + a.  A-half (l<160) fills
    hT k-tiles 0..9, B-half (l>=160) fills k-tiles 10..17."""
    ks = np.empty(S, np.int64)
    aa = np.empty(S, np.int64)
    l = np.arange(160)
    ks[:160] = l % 10
    aa[:160] = l // 10
    jj = np.arange(128)
    ks[160:] = 10 + jj % 8
    aa[160:] = jj // 8
    return ks, aa


def _col_rows(core):
    """MM column c -> row index into the [9216] gate-row dimension."""
    ks, aa = _unit_map()
    u = 128 * ks + 16 * core + aa
    rows = np.empty(G4, np.int64)
    for g in range(3):
        w, cbase, lb = GW[g], GBASE[g], LBASE[g]
        for p in range(4):
            rows[cbase + p * w: cbase + (p + 1) * w] = GO[p] * HW + u[lb:lb + w]
    return rows, u


def _sel_matrix():
    sel = np.zeros((128, T, NSEQ), np.float32)
    for s in range(T):
        for m in range(NSEQ):
            if m < 4:
                n = 8 * s + m
            elif m < 8:
                n = 8 * (15 - s) + (m - 4)
            elif m < 12:
                n = 8 * s + 4 + (m - 8)
            else:
                n = 8 * (15 - s) + 4 + (m - 12)
            sel[n, s, m] = 1.0
    return sel


def _prep_inputs(x, W_ih, W_hh, b_ih, b_hh):
    bf = ml_dtypes.bfloat16
    xr = np.asarray(x, np.float32).reshape(B, T, HW)
    Xc = xr[SAMP_ORDER].transpose(1, 0, 2).reshape(B * T, HW)
    xT = np.zeros((KTA * 128, 128), np.float32)
    xT[:HW] = Xc.T
    xT[HW] = 1.0
    xT = np.ascontiguousarray(
        xT.reshape(KTA, 128, 128).transpose(1, 0, 2)
    ).astype(bf)
    sel = _sel_matrix().astype(bf)
    eye = np.eye(16, dtype=np.float32).astype(bf)
    bias = (b_ih + b_hh).astype(np.float32)

    in_maps = []
    for core in range(NC):
        rows, _ = _col_rows(core)
        whh = np.ascontiguousarray(
            W_hh[rows].T.reshape(KT, 128, G4).transpose(1, 0, 2)
        ).astype(bf)
        wih = np.zeros((KTA * 128, G4), np.float32)
        wih[:HW] = W_ih[rows].T
        wih[HW] = bias[rows]
        wih = np.ascontiguousarray(
            wih.reshape(KTA, 128, G4).transpose(1, 0, 2)
        ).astype(bf)
        in_maps.append(
            {"xT": xT, "wih": wih, "whh": whh, "sel": sel, "eye16": eye}
        )
    return in_maps


def run(x, W_ih, W_hh, b_ih, b_hh, conv_w, conv_b, trace=False, tmpdir=None):
    w0 = float(np.asarray(conv_w).reshape(2)[0])
    w1 = float(np.asarray(conv_w).reshape(2)[1])
    cb = float(np.asarray(conv_b).reshape(1)[0])
    nc = _build(w0, w1, cb)
    in_maps = _prep_inputs(
        np.asarray(x), np.asarray(W_ih), np.asarray(W_hh),
        np.asarray(b_ih), np.asarray(b_hh),
    )
    res = bass_utils.run_bass_kernel_spmd(
        nc, in_maps, core_ids=list(range(NC)), trace=trace, tmpdir=tmpdir
    )
    y = np.empty((B, HW), dtype=np.float32)
    ks, aa = _unit_map()
    for core in range(NC):
        u = 128 * ks + 16 * core + aa
        y[:, u] = res.results[core]["out"]
    return y.reshape(B, 1, H, W).astype(np.float32), res


def kernel(x, W_ih, W_hh, b_ih, b_hh, conv_w, conv_b):
    y, _ = run(x, W_ih, W_hh, b_ih, b_hh, conv_w, conv_b, trace=False)
    return y


# revision 15
# speedup vs baseline: 1.0166x; 1.0166x over previous
"""BiConvLSTM kernel for one TRN2 chip (8 NeuronCores) — v2.

Strategy (8-way model parallelism over LSTM gate rows, pipelined AllGather):
  - Each core owns 288 hidden units; W_ih/W_hh column-slices live in SBUF.
  - All 4 gates are packed into wide moving operands: per k-tile the gate
    matmul is 3 MMs of N=512/512/128 (unit-groups of 128/128/32 units with
    gate column order i|f|o|g), instead of 4x N=288 -> fewer LDWEIGHTS
    stalls, near the bf16 streaming rate of the PE.
  - The x-projection (+bias) for all 128 (t, sample) columns is computed
    once; a per-step 0/1 selector matrix SEL_s folds the right rows into
    each gate PSUM as one extra matmul (no DVE add, no staging DMAs).
  - Hidden units are k-mapped: units of half A land in hT k-tiles 0..7,
    half B in k-tiles 8..17.  Each step runs TWO AllGathers (A then B);
    the next step's matmuls consume k 0..7 first, so AG_B may land ~3.4us
    into the next step.  AG latency hides under real matmuls (no dummies).
  - Epilogue: one sigmoid over [i|f|o], one tanh for g, c/h updates on DVE;
    h is produced in bf16, PE-transposed straight into a bf16 PSUM tile and
    DMA'd to the collective input buffer.

Sequence-row layout (row m of the 16-row state tiles):
  m 0..3   fwd samples 0,2,4,6;  m 4..7   bwd samples 0,2,4,6
  m 8..11  fwd samples 1,3,5,7;  m 12..15 bwd samples 1,3,5,7
This makes the reference's stack/reshape channel pairing (row b', b'+8).
"""

import sys

if "/opt/trn_rl_repo" not in sys.path:
    sys.path.append("/opt/trn_rl_repo")

import ml_dtypes
import numpy as np

from concourse import bacc, bass_utils, mybir, tile
from concourse.tile import add_dep_helper

B, T, H, W = 8, 16, 48, 48
HW = H * W              # 2304
NSEQ = 16               # 8 samples x 2 directions
NC = 8                  # cores
S = HW // NC            # 288 hidden units per core
G4 = 4 * S              # 1152 gate columns per core
KT = HW // 128          # 18 k-tiles
KTA = KT + 1            # +1 tile holding the bias/ones row
GO = [0, 1, 3, 2]       # column-position -> gate (i, f, o, g)
GW = [128, 32, 128]     # unit-group widths (A=128+32 units, B=128)
GBASE = [0, 512, 640]   # gate-column base per group
LBASE = [0, 128, 160]   # local-unit base per group
SAMP_ORDER = [0, 2, 4, 6, 1, 3, 5, 7]
F32 = mybir.dt.float32
BF16 = mybir.dt.bfloat16
RG = [list(range(NC))]


def _build(w0: float, w1: float, cb: float):
    nc = bacc.Bacc("TRN2", target_bir_lowering=False, debug=False, num_devices=NC)

    xT_d = nc.dram_tensor("xT", [128, KTA, 128], BF16, kind="ExternalInput")
    wih_d = nc.dram_tensor("wih", [128, KTA, G4], BF16, kind="ExternalInput")
    whh_d = nc.dram_tensor("whh", [128, KT, G4], BF16, kind="ExternalInput")
    sel_d = nc.dram_tensor("sel", [128, T, NSEQ], BF16, kind="ExternalInput")
    eye_d = nc.dram_tensor("eye16", [16, 16], BF16, kind="ExternalInput")
    out_d = nc.dram_tensor("out", [B, S], F32, kind="ExternalOutput")

    SIG = mybir.ActivationFunctionType.Sigmoid
    TANH = mybir.ActivationFunctionType.Tanh
    ADD = mybir.AluOpType.add
    MULT = mybir.AluOpType.mult
    MAX = mybir.AluOpType.max

    with tile.TileContext(nc) as tc:
        with (
            tc.tile_pool(name="const", bufs=1) as constp,
            tc.tile_pool(name="state", bufs=2) as statep,
            tc.tile_pool(name="ew", bufs=2) as ewp,
            tc.tile_pool(name="pg", bufs=2, space="PSUM") as pgp,
            tc.tile_pool(name="pt", bufs=1, space="PSUM") as ptp,
            tc.tile_pool(name="px", bufs=1, space="PSUM") as pxp,
            tc.tile_pool(name="dram", bufs=2, space="DRAM") as dp,
        ):
            # throwaway collective: absorbs the ncfw first-call warmup while
            # the weight DMAs stream in
            warm_in = dp.tile([16, NSEQ], BF16, tag="warmin")
            warm_out = dp.tile([128, NSEQ], BF16, addr_space="Shared", tag="warmout")
            warm_ag = nc.gpsimd.collective_compute(
                "AllGather", mybir.AluOpType.bypass,
                ins=[warm_in.opt()], outs=[warm_out.opt()], replica_groups=RG,
            )

            eye_sb = constp.tile([16, 16], BF16, tag="eye")
            nc.scalar.dma_start(eye_sb[:, :], eye_d[:, :])
            sel_sb = constp.tile([128, T, NSEQ], BF16, tag="sel")
            nc.scalar.dma_start(sel_sb[:, :, :], sel_d[:, :, :])
            # touch sigmoid/tanh so the ACT tables load off the critical path
            actwarm = constp.tile([16, 16], F32, tag="actwarm")
            nc.scalar.activation(actwarm[:, :], eye_sb[:, :], SIG)
            nc.scalar.activation(actwarm[:, :], eye_sb[:, :], TANH)

            xT_sb = constp.tile([128, KTA, 128], BF16, tag="xT")
            nc.sync.dma_start(xT_sb[:, 0:10, :], xT_d[:, 0:10, :])
            nc.scalar.dma_start(xT_sb[:, 10:KTA, :], xT_d[:, 10:KTA, :])

            # stream W_ih (k-ascending; xproj chases these), then W_hh
            wih_sb = constp.tile([128, KTA, G4], BF16, tag="wih")
            whh_sb = constp.tile([128, KT, G4], BF16, tag="whh")
            # gpsimd carries ONLY collectives: any weight DMA queued there can
            # get scheduled ahead of the warmup AG's doorbell and delay the
            # ~25-50us ncfw init that gates step 0's first real AllGather
            qs = [nc.sync, nc.scalar]
            wih_chunks = [(k, min(k + 2, KTA)) for k in range(0, KTA, 2)]
            for i, (k0, k1) in enumerate(wih_chunks):
                qs[i % 2].dma_start(wih_sb[:, k0:k1, :], wih_d[:, k0:k1, :])
            def stream_whh():
                # emitted at the END of step 0's body: keeps these issue-waits
                # behind step 0's scalar/sync work in the engine queues (whh
                # is only consumed from step 1 onward)
                whh_chunks = [(k, min(k + 2, KT)) for k in range(0, KT, 2)]
                for i, (k0, k1) in enumerate(whh_chunks):
                    qs[i % 2].dma_start(whh_sb[:, k0:k1, :], whh_d[:, k0:k1, :])

            # PE instruction chain: explicit deps pin the PE issue order
            prev_pe = None

            def pe(instr):
                nonlocal prev_pe
                if prev_pe is not None:
                    add_dep_helper(instr.ins, prev_pe.ins, False, reason="pe order")
                prev_pe = instr
                return instr

            # ---- xproj[n, c] = [xT; ones].T @ [W_ih.T; bias], all 128 cols
            xproj_bf = constp.tile([128, G4], BF16, tag="xproj")
            px = pxp.tile([128, 512], F32, tag="px")
            for g in range(3):
                cs = slice(GBASE[g], GBASE[g] + 4 * GW[g])
                for k in range(KTA):
                    pe(nc.tensor.matmul(
                        px[:, 0: 4 * GW[g]],
                        lhsT=xT_sb[:, k, :],
                        rhs=wih_sb[:, k, cs],
                        start=(k == 0),
                        stop=(k == KTA - 1),
                    ))
                nc.vector.tensor_copy(xproj_bf[:, cs], px[:, 0: 4 * GW[g]])

            c_prev = None
            h_last = None
            hT_prev = None

            def alloc_pg(s):
                return [
                    pgp.tile([NSEQ, 4 * GW[g]], F32, tag=f"pg{g}", name=f"pg{g}_{s}")
                    for g in range(3)
                ]

            def xsel_mms(pg, s):
                for g in range(3):
                    cs = slice(GBASE[g], GBASE[g] + 4 * GW[g])
                    pe(nc.tensor.matmul(
                        pg[g][:, :], lhsT=sel_sb[:, s, :], rhs=xproj_bf[:, cs],
                        start=True, stop=(s == 0),
                    ))

            def epi(pg, g, s, c_new, h_new):
                w = GW[g]
                lb = LBASE[g]
                sg = ewp.tile([NSEQ, 3 * w], F32, tag=f"sg{g}", name=f"sg{g}_{s}")
                tg = ewp.tile([NSEQ, w], F32, tag=f"tg{g}", name=f"tg{g}_{s}")
                nc.scalar.activation(tg[:, :], pg[g][:, 3 * w: 4 * w], TANH)
                nc.scalar.activation(sg[:, :], pg[g][:, 0: 3 * w], SIG)
                cslc = c_new[:, lb: lb + w]
                if s == 0:
                    nc.vector.tensor_tensor(cslc, sg[:, 0:w], tg[:, :], MULT)
                else:
                    m1 = ewp.tile([NSEQ, w], F32, tag=f"m1{g}", name=f"m1{g}_{s}")
                    fc = ewp.tile([NSEQ, w], F32, tag=f"fc{g}", name=f"fc{g}_{s}")
                    nc.vector.tensor_tensor(m1[:, :], sg[:, 0:w], tg[:, :], MULT)
                    nc.vector.tensor_tensor(
                        fc[:, :], sg[:, w: 2 * w], c_prev[:, lb: lb + w], MULT
                    )
                    nc.vector.tensor_tensor(cslc, fc[:, :], m1[:, :], ADD)
                tc_ = ewp.tile([NSEQ, w], F32, tag=f"tc{g}", name=f"tc{g}_{s}")
                nc.scalar.activation(tc_[:, :], cslc, TANH)
                nc.vector.tensor_tensor(
                    h_new[:, lb: lb + w], sg[:, 2 * w: 3 * w], tc_[:, :], MULT
                )

            # s=0 selector matmuls (gates = xproj only)
            pg_cur = alloc_pg(0)
            xsel_mms(pg_cur, 0)

            for s in range(T):
                pg = pg_cur
                c_new = statep.tile([NSEQ, S], F32, tag="c", name=f"c{s}")
                h_new = statep.tile([NSEQ, S], BF16, tag="h", name=f"h{s}")
                csl = [slice(GBASE[g], GBASE[g] + 4 * GW[g]) for g in range(3)]

                if s > 0:
                    # phase1: all groups' k0..9 (needs only AG_A of step s-1);
                    # long enough to cover AG_B's later arrival
                    for g in range(3):
                        for k in range(10):
                            pe(nc.tensor.matmul(
                                pg[g][:, :], lhsT=hT_prev[:, k, :],
                                rhs=whh_sb[:, k, csl[g]], start=False, stop=False,
                            ))
                    # phase2: k10..17, A-half groups first so AG_A goes early
                    for g in range(3):
                        for k in range(10, KT):
                            pe(nc.tensor.matmul(
                                pg[g][:, :], lhsT=hT_prev[:, k, :],
                                rhs=whh_sb[:, k, csl[g]], start=False,
                                stop=(k == KT - 1),
                            ))
                        epi(pg, g, s, c_new, h_new)
                else:
                    for g in range(3):
                        epi(pg, g, s, c_new, h_new)
                c_prev = c_new
                h_last = h_new

                if s < T - 1:
                    tp = ptp.tile([128, 48], BF16, tag="tp", name=f"tp{s}")
                    hts = ewp.tile([128, 48], BF16, tag="hts", name=f"hts{s}")
                    ccinA = dp.tile([160, NSEQ], BF16, tag="ccinA", name=f"ccinA{s}")
                    ccinB = dp.tile([128, NSEQ], BF16, tag="ccinB", name=f"ccinB{s}")
                    pe(nc.tensor.transpose(tp[:, 0:16], h_new[:, 0:128], eye_sb[:, :]))
                    nc.vector.tensor_copy(hts[:, 0:16], tp[:, 0:16])
                    nc.sync.dma_start(ccinA[0:128, :], hts[:, 0:16])
                    pe(nc.tensor.transpose(tp[0:32, 16:32], h_new[:, 128:160], eye_sb[:, :]))
                    nc.vector.tensor_copy(hts[0:32, 16:32], tp[0:32, 16:32])
                    nc.scalar.dma_start(ccinA[128:160, :], hts[0:32, 16:32])
                    pe(nc.tensor.transpose(tp[:, 32:48], h_new[:, 160:288], eye_sb[:, :]))
                    nc.vector.tensor_copy(hts[:, 32:48], tp[:, 32:48])
                    nc.sync.dma_start(ccinB[:, :], hts[:, 32:48])
                    ccoutA = dp.tile(
                        [128, 10, NSEQ], BF16, addr_space="Shared",
                        tag="ccoutA", name=f"ccoutA{s}",
                    )
                    ccoutB = dp.tile(
                        [128, 8, NSEQ], BF16, addr_space="Shared",
                        tag="ccoutB", name=f"ccoutB{s}",
                    )
                    agA = nc.gpsimd.collective_compute(
                        "AllGather", mybir.AluOpType.bypass,
                        ins=[ccinA.opt()], outs=[ccoutA.opt()], replica_groups=RG,
                    )
                    agB = nc.gpsimd.collective_compute(
                        "AllGather", mybir.AluOpType.bypass,
                        ins=[ccinB.opt()], outs=[ccoutB.opt()], replica_groups=RG,
                    )
                    add_dep_helper(agB.ins, agA.ins, False, reason="AG order")
                    # next step's selector matmuls + warm-keeper dummies fill
                    # the PE while the AllGathers run
                    pg_cur = alloc_pg(s + 1)
                    xsel_mms(pg_cur, s + 1)
                    for di in range(12):
                        pe(nc.tensor.matmul(
                            px[0:NSEQ, 0:512], lhsT=sel_sb[:, s, :],
                            rhs=xproj_bf[:, 0:512], start=True, stop=True,
                        ))
                    hT_new = statep.tile([128, KT, NSEQ], BF16, tag="hT", name=f"hT{s}")
                    nc.sync.dma_start(hT_new[:, 0:10, :], ccoutA[:, :, :])
                    nc.sync.dma_start(hT_new[:, 10:14, :], ccoutB[:, 0:4, :])
                    nc.sync.dma_start(hT_new[:, 14:KT, :], ccoutB[:, 4:8, :])
                    hT_prev = hT_new
                    if s == 0:
                        stream_whh()

            # ---- epilogue: y[b'] = leaky(w0*tanh(h[b']) + w1*tanh(h[b'+8]) + cb)
            # move the bwd rows down FIRST (bf16, before any tanh) so the
            # cross-partition DMA latency overlaps the fwd-row tanh
            hb = ewp.tile([B, S], BF16, tag="hb")
            nc.sync.dma_start(hb[:, :], h_last[8:16, :])
            th = ewp.tile([B, S], F32, tag="th")
            nc.scalar.activation(th[:, :], h_last[0:8, :], TANH)
            thb = ewp.tile([B, S], F32, tag="thb")
            nc.scalar.activation(thb[:, :], hb[:, :], TANH)
            yb = ewp.tile([B, S], F32, tag="yb")
            nc.vector.tensor_scalar(yb[:, :], thb[:, :], w1, cb, MULT, ADD)
            yc = ewp.tile([B, S], F32, tag="yc")
            nc.vector.scalar_tensor_tensor(yc[:, :], th[:, :], w0, yb[:, :], MULT, ADD)
            ye = ewp.tile([B, S], F32, tag="ye")
            nc.vector.scalar_tensor_tensor(ye[:, :], yc[:, :], 0.01, yc[:, :], MULT, MAX)
            nc.sync.dma_start(out_d[:, :], ye[:, :])

    nc.compile()
    return nc


# ---------------- host-side layout prep ----------------

def _unit_map():
    """local unit l -> (k, a); u = 128*k + 16*r + a.  A-half (l<160) fills
    hT k-tiles 0..9, B-half (l>=160) fills k-tiles 10..17."""
    ks = np.empty(S, np.int64)
    aa = np.empty(S, np.int64)
    l = np.arange(160)
    ks[:160] = l % 10
    aa[:160] = l // 10
    jj = np.arange(128)
    ks[160:] = 10 + jj % 8
    aa[160:] = jj // 8
    return ks, aa


def _col_rows(core):
    """MM column c -> row index into the [9216] gate-row dimension."""
    ks, aa = _unit_map()
    u = 128 * ks + 16 * core + aa
    rows = np.empty(G4, np.int64)
    for g in range(3):
        w, cbase, lb = GW[g], GBASE[g], LBASE[g]
        for p in range(4):
            rows[cbase + p * w: cbase + (p + 1) * w] = GO[p] * HW + u[lb:lb + w]
    return rows, u


def _sel_matrix():
    sel = np.zeros((128, T, NSEQ), np.float32)
    for s in range(T):
        for m in range(NSEQ):
            if m < 4:
                n = 8 * s + m
            elif m < 8:
                n = 8 * (15 - s) + (m - 4)
            elif m < 12:
                n = 8 * s + 4 + (m - 8)
            else:
                n = 8 * (15 - s) + 4 + (m - 12)
            sel[n, s, m] = 1.0
    return sel


def _prep_inputs(x, W_ih, W_hh, b_ih, b_hh):
    bf = ml_dtypes.bfloat16
    xr = np.asarray(x, np.float32).reshape(B, T, HW)
    Xc = xr[SAMP_ORDER].transpose(1, 0, 2).reshape(B * T, HW)
    xT = np.zeros((KTA * 128, 128), np.float32)
    xT[:HW] = Xc.T
    xT[HW] = 1.0
    xT = np.ascontiguousarray(
        xT.reshape(KTA, 128, 128).transpose(1, 0, 2)
    ).astype(bf)
    sel = _sel_matrix().astype(bf)
    eye = np.eye(16, dtype=np.float32).astype(bf)
    bias = (b_ih + b_hh).astype(np.float32)

    in_maps = []
    for core in range(NC):
        rows, _ = _col_rows(core)
        whh = np.ascontiguousarray(
            W_hh[rows].T.reshape(KT, 128, G4).transpose(1, 0, 2)
        ).astype(bf)
        wih = np.zeros((KTA * 128, G4), np.float32)
        wih[:HW] = W_ih[rows].T
        wih[HW] = bias[rows]
        wih = np.ascontiguousarray(
            wih.reshape(KTA, 128, G4).transpose(1, 0, 2)
        ).astype(bf)
        in_maps.append(
            {"xT": xT, "wih": wih, "whh": whh, "sel": sel, "eye16": eye}
        )
    return in_maps


def run(x, W_ih, W_hh, b_ih, b_hh, conv_w, conv_b, trace=False, tmpdir=None):
    w0 = float(np.asarray(conv_w).reshape(2)[0])
    w1 = float(np.asarray(conv_w).reshape(2)[1])
    cb = float(np.asarray(conv_b).reshape(1)[0])
    nc = _build(w0, w1, cb)
    in_maps = _prep_inputs(
        np.asarray(x), np.asarray(W_ih), np.asarray(W_hh),
        np.asarray(b_ih), np.asarray(b_hh),
    )
    res = bass_utils.run_bass_kernel_spmd(
        nc, in_maps, core_ids=list(range(NC)), trace=trace, tmpdir=tmpdir
    )
    y = np.empty((B, HW), dtype=np.float32)
    ks, aa = _unit_map()
    for core in range(NC):
        u = 128 * ks + 16 * core + aa
        y[:, u] = res.results[core]["out"]
    return y.reshape(B, 1, H, W).astype(np.float32), res


def kernel(x, W_ih, W_hh, b_ih, b_hh, conv_w, conv_b):
    y, _ = run(x, W_ih, W_hh, b_ih, b_hh, conv_w, conv_b, trace=False)
    return y
